# revision 43
# baseline (speedup 1.0000x reference)
"""YOLOv3-style detection decode kernel for Trainium2 (8 NeuronCores).

Data-parallel over batch (32 -> 4 per core). The host packs ALL THREE
scales' cells into one stream (169+676+2704 = 3549 cells -> 28 blocks of
128) in a cells-on-partitions layout x[p, (b k a c)] (c = 85 attrs per
anchor; 3*85 = 255 channels, so (b k a) collapse into one free dim).
Per-cell constants (grid offsets, stride, anchors) become per-(p,k) const
tensors, so the device runs ONE fused pass over everything:

  - argmax over the 80 classes per (cell, anchor) via two tensor_tensor
    fold tournaments: phase-maxes p8[j] = max_g x[8g+j] and group-maxes
    q10[g] = max_j x[8g+j]; index = 8g* + j* recovered by an is_ge
    compare against the max plus a descending-weight max (ties break
    toward the FIRST index, matching jnp.argmax).
  - conf mask on ACT (relu(sign(conf-thr))); box decode from strided
    views of the same tiles (exp on ACT).
  - output packed [p, b, k, a, 6]; the host re-interleaves per scale.
"""

import sys

import numpy as np

if "/opt/trn_rl_repo" not in sys.path:
    sys.path.insert(0, "/opt/trn_rl_repo")

NUM_ATTRS = 85
B_LOC = 4  # batches per core (32 / 8)
N_CORES = 8

# (name, H, stride) — cells are packed in this order
_SCALES = (
    ("13", 13, 32.0),
    ("26", 26, 16.0),
    ("52", 52, 8.0),
)
HW_TOT = sum(H * H for _, H, _ in _SCALES)       # 3549
NBLK = -(-HW_TOT // 128)                         # 28 blocks
ZB = NBLK * 3                                    # z-units per batch (84)
Z = B_LOC * ZB                                   # 336
XCOLS = B_LOC * NBLK * 255

# consts layout: w8(8) | w10(10) | negthr(1) | gx4 | gy4 | sf4 | anch
_CST_W8 = 0
_CST_W10 = 8
_CST_THR = 18
_CST_GX = 19
_CST_GY = _CST_GX + B_LOC * NBLK
_CST_SF = _CST_GY + B_LOC * NBLK
_CST_AN = _CST_SF + B_LOC * NBLK
CST_COLS = _CST_AN + NBLK * 6


def _build_program():
    import concourse.bass as bass
    import concourse.mybir as mybir
    from concourse.tile import TileContext

    f32 = mybir.dt.float32
    bf16 = mybir.dt.bfloat16
    Alu = mybir.AluOpType
    Act = mybir.ActivationFunctionType

    nc = bass.Bass(trn_type="TRN2")

    xin = nc.declare_dram_parameter("x", [128, XCOLS], f32, False)
    # conf+boxes ship as soon as the box epilogue finishes (overlapping the
    # argmax extraction); the small cls tensor ships last.
    opk = nc.declare_dram_parameter("opack", [128, Z * 5], f32, True)
    opc = nc.declare_dram_parameter("opcls", [128, Z], f32, True)
    cst_p = nc.declare_dram_parameter("cst", [128, CST_COLS], f32, False)

    with TileContext(nc) as tc:
        from contextlib import ExitStack
        with ExitStack() as ctx:
            cpool = ctx.enter_context(tc.tile_pool(name="consts", bufs=1))
            xpool = ctx.enter_context(tc.tile_pool(name="xb", bufs=2))
            # fold scratch: consumed only by the (serial) DVE queue
            fpool = ctx.enter_context(tc.tile_pool(name="folds", bufs=1))

            # consts dispatch on the ACT HWDGE ring so the first input DMA
            # is the very first thing on the sync ring
            cstt = cpool.tile([128, CST_COLS], f32, tag="cst", name="cstt")
            nc.scalar.dma_start(out=cstt[:, :], in_=cst_p[:, :])
            thr = cstt[:, _CST_THR:_CST_THR + 1]

            w8c = cpool.tile([128, 8], bf16, tag="w8c", name="w8c")
            w10c = cpool.tile([128, 10], bf16, tag="w10c", name="w10c")

            t = {}
            for key, w in (("p8", 8), ("q10", 10), ("m", 1), ("ts", 1),
                           ("mask", 1), ("ex", 2), ("wh", 2), ("cx", 1),
                           ("cy", 1), ("opk", 5), ("opc", 1), ("m2", 4), ("bx", 4)):
                t[key] = cpool.tile([128, Z * w], f32, tag=key, name=key)
            for key, w in (("eq8", 8), ("eq10", 10), ("ew8", 8),
                           ("ew10", 10), ("r8", 1), ("r10", 1),
                           ("rs8", 4), ("rs10", 5)):
                t[key] = cpool.tile([128, Z * w], bf16, tag=key, name=key)

            def class_reduces(xtv, lo):
                """Folds + conf/mask + cx/cy for one [p, z, 85] view at z
                offset lo (within a batch: gx/sf consts are b-replicated)."""
                zc = xtv.shape[1]
                hi = lo + zc

                cls = xtv[:, :, 5:85]
                p8v = t["p8"][:, lo * 8:hi * 8].rearrange(
                    "p (z j) -> p z j", j=8)

                # phase-max p8[j] = max_g cls[8g+j]: fold the group axis.
                f1 = fpool.tile([128, zc * 40], f32, tag="f1", name="f1")
                f1v = f1[:, :].rearrange("p (z c) -> p z c", c=40)
                nc.vector.tensor_tensor(out=f1v, in0=cls[:, :, 0:40],
                                        in1=cls[:, :, 40:80], op=Alu.max)
                f2 = fpool.tile([128, zc * 16], f32, tag="f2", name="f2")
                f2v = f2[:, :].rearrange("p (z c) -> p z c", c=16)
                nc.vector.tensor_tensor(out=f2v, in0=f1v[:, :, 0:16],
                                        in1=f1v[:, :, 16:32], op=Alu.max)
                f3 = fpool.tile([128, zc * 8], f32, tag="f3", name="f3")
                f3v = f3[:, :].rearrange("p (z c) -> p z c", c=8)
                nc.vector.tensor_tensor(out=f3v, in0=f2v[:, :, 0:8],
                                        in1=f2v[:, :, 8:16], op=Alu.max)
                nc.vector.tensor_tensor(out=p8v, in0=f3v,
                                        in1=f1v[:, :, 32:40], op=Alu.max)
                # group-max q10[g] = max_j cls[8g+j]: fold within groups.
                cg = cls.rearrange("p z (g j) -> p z g j", g=10, j=8)
                y1 = fpool.tile([128, zc * 40], f32, tag="y1", name="y1")
                y1v = y1[:, :].rearrange("p (z g j) -> p z g j", g=10, j=4)
                nc.vector.tensor_tensor(out=y1v, in0=cg[:, :, :, 0:4],
                                        in1=cg[:, :, :, 4:8], op=Alu.max)
                y1z = y1[:, :].rearrange("p (w j) -> p w j", j=4)
                y2 = fpool.tile([128, zc * 20], f32, tag="y2", name="y2")
                y2z = y2[:, :].rearrange("p (w j) -> p w j", j=2)
                nc.vector.tensor_tensor(out=y2z, in0=y1z[:, :, 0:2],
                                        in1=y1z[:, :, 2:4], op=Alu.max)
                nc.vector.tensor_tensor(
                    out=t["q10"][:, lo * 10:hi * 10],
                    in0=y2z[:, :, 0:1].squeeze(2),
                    in1=y2z[:, :, 1:2].squeeze(2), op=Alu.max)

                # mask = 1[conf > thr] on ACT: relu(sign(conf + negthr))
                conf = xtv[:, :, 0:1].squeeze(2)
                maskv = t["mask"][:, lo:hi]
                sgn = t["m2"][:, lo:hi]
                nc.scalar.activation(out=sgn, in_=conf, func=Act.Sign,
                                     bias=thr)
                nc.scalar.activation(out=maskv, in_=sgn, func=Act.Relu)
                opkz = t["opk"][:, lo * 5:hi * 5].rearrange(
                    "p (z q) -> p z q", q=5)
                nc.vector.tensor_tensor(
                    out=opkz[:, :, 0:1].squeeze(2), in0=conf, in1=maskv,
                    op=Alu.mult)

                # exp(tw,th) on ACT; cx/cy = t*stride + grid on DVE
                # (stride varies per cell now -> tensor_tensor with sf4)
                exv = t["ex"][:, lo * 2:hi * 2].rearrange(
                    "p (z e) -> p z e", e=2)
                nc.scalar.activation(out=exv, in_=xtv[:, :, 3:5],
                                     func=Act.Exp)
                gx = cstt[:, _CST_GX + lo // 3:_CST_GX + hi // 3]
                gy = cstt[:, _CST_GY + lo // 3:_CST_GY + hi // 3]
                sf = cstt[:, _CST_SF + lo // 3:_CST_SF + hi // 3]
                gxb = gx.unsqueeze(2).broadcast_to([128, zc // 3, 3])
                gyb = gy.unsqueeze(2).broadcast_to([128, zc // 3, 3])
                sfb = sf.unsqueeze(2).broadcast_to([128, zc // 3, 3])
                tx = xtv[:, :, 1:2].squeeze(2).rearrange(
                    "p (w a) -> p w a", a=3)
                ty = xtv[:, :, 2:3].squeeze(2).rearrange(
                    "p (w a) -> p w a", a=3)
                cxv = t["cx"][:, lo:hi].rearrange("p (w a) -> p w a", a=3)
                cyv = t["cy"][:, lo:hi].rearrange("p (w a) -> p w a", a=3)
                sc = t["bx"][:, lo:hi].rearrange("p (w a) -> p w a", a=3)
                nc.vector.tensor_tensor(out=sc, in0=tx, in1=sfb, op=Alu.mult)
                nc.vector.tensor_tensor(out=cxv, in0=sc, in1=gxb, op=Alu.add)
                nc.vector.tensor_tensor(out=sc, in0=ty, in1=sfb, op=Alu.mult)
                nc.vector.tensor_tensor(out=cyv, in0=sc, in1=gyb, op=Alu.add)

            def epilogue():
                """Argmax extraction + boxes over all batches at once."""
                def zv(tile, w):
                    return tile[:, :].rearrange("p (z q) -> p z q", q=w)

                p8 = zv(t["p8"], 8)
                q10 = zv(t["q10"], 10)
                eq8 = zv(t["eq8"], 8)
                eq10 = zv(t["eq10"], 10)
                ew8 = zv(t["ew8"], 8)
                ew10 = zv(t["ew10"], 10)
                m = t["m"][:, :]
                r8 = t["r8"][:, :]
                r10 = t["r10"][:, :]
                tsv = t["ts"][:, :]
                mask = t["mask"][:, :]
                opkq = zv(t["opk"], 5)

                # boxes: wh = anch * exp (anch varies per cell -> one TT
                # over all batches), then x1/y1/x2/y2, then masked write.
                anch = cstt[:, _CST_AN:_CST_AN + NBLK * 6]
                anb = anch.unsqueeze(1).broadcast_to([128, B_LOC, NBLK * 6])
                exb = t["ex"][:, :].rearrange("p (b e) -> p b e",
                                              b=B_LOC)
                whb = t["wh"][:, :].rearrange("p (b e) -> p b e",
                                              b=B_LOC)
                nc.vector.tensor_tensor(out=whb, in0=exb, in1=anb,
                                        op=Alu.mult)
                whz = zv(t["wh"], 2)
                wx = whz[:, :, 0:1].squeeze(2)
                wy = whz[:, :, 1:2].squeeze(2)
                cx = t["cx"][:, :]
                cy = t["cy"][:, :]
                bx = zv(t["bx"], 4)
                for q, (wv, cv, s) in enumerate(
                        ((wx, cx, -0.5), (wy, cy, -0.5),
                         (wx, cx, 0.5), (wy, cy, 0.5))):
                    nc.vector.scalar_tensor_tensor(
                        out=bx[:, :, q:q + 1].squeeze(2), in0=wv, scalar=s,
                        in1=cv, op0=Alu.mult, op1=Alu.add)
                mb4 = mask.unsqueeze(2).broadcast_to([128, Z, 4])
                nc.vector.tensor_tensor(out=opkq[:, :, 1:5], in0=bx,
                                        in1=mb4, op=Alu.mult)
                nc.sync.dma_start(out=opk[:, :], in_=t["opk"][:, :])
                # m = max over phases via TT folds (8 -> 4 -> 2 -> 1)
                m2 = zv(t["m2"], 4)
                nc.vector.tensor_tensor(out=m2, in0=p8[:, :, 0:4],
                                        in1=p8[:, :, 4:8], op=Alu.max)
                m2z = t["m2"][:, :].rearrange("p (w c) -> p w c", c=2)
                nc.vector.tensor_tensor(out=m2z[:, :, 0:1].squeeze(2),
                                        in0=m2z[:, :, 0:1].squeeze(2),
                                        in1=m2z[:, :, 1:2].squeeze(2),
                                        op=Alu.max)
                nc.vector.tensor_tensor(out=m, in0=m2[:, :, 0:1].squeeze(2),
                                        in1=m2[:, :, 2:3].squeeze(2),
                                        op=Alu.max)
                mb8 = m.unsqueeze(2).broadcast_to([128, Z, 8])
                mb10 = m.unsqueeze(2).broadcast_to([128, Z, 10])
                w8b = w8c[:, :].unsqueeze(1).broadcast_to([128, Z, 8])
                w10b = w10c[:, :].unsqueeze(1).broadcast_to([128, Z, 10])

                # j*/g* via descending-weight max (first-index tie-break)
                nc.vector.tensor_tensor(out=eq8, in0=p8, in1=mb8,
                                        op=Alu.is_ge)
                nc.vector.tensor_tensor(out=ew8, in0=eq8, in1=w8b,
                                        op=Alu.mult)
                rs8 = zv(t["rs8"], 4)
                nc.vector.tensor_tensor(out=rs8, in0=ew8[:, :, 0:4],
                                        in1=ew8[:, :, 4:8], op=Alu.max)
                rs8z = t["rs8"][:, :].rearrange("p (w c) -> p w c", c=2)
                nc.vector.tensor_tensor(out=rs8z[:, :, 0:1].squeeze(2),
                                        in0=rs8z[:, :, 0:1].squeeze(2),
                                        in1=rs8z[:, :, 1:2].squeeze(2),
                                        op=Alu.max)
                nc.vector.tensor_tensor(out=r8,
                                        in0=rs8[:, :, 0:1].squeeze(2),
                                        in1=rs8[:, :, 2:3].squeeze(2),
                                        op=Alu.max)
                nc.vector.tensor_tensor(out=eq10, in0=q10, in1=mb10,
                                        op=Alu.is_ge)
                nc.vector.tensor_tensor(out=ew10, in0=eq10, in1=w10b,
                                        op=Alu.mult)
                rs10 = zv(t["rs10"], 5)
                nc.vector.tensor_tensor(out=rs10, in0=ew10[:, :, 0:5],
                                        in1=ew10[:, :, 5:10], op=Alu.max)
                nc.vector.tensor_tensor(out=rs10[:, :, 0:2],
                                        in0=rs10[:, :, 0:2],
                                        in1=rs10[:, :, 2:4], op=Alu.max)
                nc.vector.tensor_tensor(out=rs10[:, :, 0:1].squeeze(2),
                                        in0=rs10[:, :, 0:1].squeeze(2),
                                        in1=rs10[:, :, 1:2].squeeze(2),
                                        op=Alu.max)
                nc.vector.tensor_tensor(out=r10,
                                        in0=rs10[:, :, 0:1].squeeze(2),
                                        in1=rs10[:, :, 4:5].squeeze(2),
                                        op=Alu.max)
                # idx = 88 - 8*r10 - r8 ; cls_m = (ts + 88) * mask
                nc.vector.scalar_tensor_tensor(
                    out=tsv, in0=r10, scalar=-8.0, in1=r8,
                    op0=Alu.mult, op1=Alu.subtract)
                nc.vector.scalar_tensor_tensor(
                    out=t["opc"][:, :], in0=tsv, scalar=88.0,
                    in1=mask, op0=Alu.add, op1=Alu.mult)
                nc.sync.dma_start(out=opc[:, :], in_=t["opc"][:, :])



            # per-batch input tiles; batch 0 lands in two halves so the
            # DVE starts as early as possible
            bc = NBLK * 255
            for b in range(B_LOC):
                xtb = xpool.tile([128, bc], f32, tag="xb", name="xtb")
                if b == 0:
                    hb = 10 * 3 * 85
                    nc.sync.dma_start(out=xtb[:, 0:hb], in_=xin[:, 0:hb])
                    nc.sync.dma_start(out=xtb[:, hb:], in_=xin[:, hb:bc])
                    z1 = 10 * 3  # leading chunk small -> DVE starts early
                    h = z1 * 85
                    v1 = xtb[:, 0:h].rearrange("p (z c) -> p z c", c=85)
                    class_reduces(v1, 0)
                    v2 = xtb[:, h:].rearrange("p (z c) -> p z c", c=85)
                    class_reduces(v2, z1)
                else:
                    nc.sync.dma_start(out=xtb[:, :],
                                      in_=xin[:, b * bc:(b + 1) * bc])
                    vz = xtb[:, :].rearrange("p (z c) -> p z c", c=85)
                    class_reduces(vz, b * ZB)
            # bf16 index weights: emitted HERE so these tiny CASTs sit
            # behind the fold work in the DVE queue instead of at its head
            # (the consts DMA lands late behind the input stream).
            nc.vector.tensor_copy(out=w8c[:, :],
                                  in_=cstt[:, _CST_W8:_CST_W8 + 8])
            nc.vector.tensor_copy(out=w10c[:, :],
                                  in_=cstt[:, _CST_W10:_CST_W10 + 10])
            epilogue()

    return nc


def _split_sync_waits(nc, limit=1):
    """Move overflow sync waits onto standalone NoOps (several instruction
    structs only have one wait slot; walrus hard-errors otherwise)."""
    import concourse.mybir as mybir

    for f in nc.m.functions:
        for blk in f.blocks:
            out = []
            changed = False
            for i in blk.instructions:
                si = i.sync_info
                tname = type(i).__name__
                if (si is not None and si.on_wait
                        and len(si.on_wait) > limit
                        and tname not in ("InstEventSemaphore",)):
                    waits = list(si.on_wait)
                    keep = waits[-limit:]
                    spill = waits[:-limit]
                    for k, w in enumerate(spill):
                        nop = mybir.InstNoOp(
                            name=f"{i.name}-sw{k}", ins=[], outs=[])
                        nop.engine = i.engine
                        nop.sync_info = mybir.SyncInfo(
                            on_wait=[w], on_update=[])
                        out.append(nop)
                    i.sync_info = mybir.SyncInfo(
                        on_wait=keep, on_update=list(si.on_update or []))
                    changed = True
                out.append(i)
            if changed:
                blk.instructions = out


_NC_CACHE = None


def _get_program(split=True):
    global _NC_CACHE
    if _NC_CACHE is None:
        _NC_CACHE = _build_program()
    if split and not getattr(_NC_CACHE, "_waits_split", False):
        _split_sync_waits(_NC_CACHE)
        _NC_CACHE._waits_split = True
    return _NC_CACHE


def _cell_consts():
    """Per-packed-cell grid/stride/anchor tables, [HW_TOT]-shaped."""
    gx = np.zeros(NBLK * 128, np.float32)
    gy = np.zeros(NBLK * 128, np.float32)
    sf = np.zeros(NBLK * 128, np.float32)
    an = np.zeros((NBLK * 128, 3, 2), np.float32)
    return gx, gy, sf, an


def _core_inputs(core, outs, anchors, threshold):
    """Build the DRAM input map for one core. Pure data marshaling."""
    xs = []
    for s_cfg, x_full in zip(_SCALES, outs):
        _, H, _ = s_cfg
        x = np.asarray(
            x_full[core * B_LOC:(core + 1) * B_LOC], dtype=np.float32
        ).reshape(B_LOC, 255, H * H)
        xs.append(x)
    xcat = np.concatenate(xs, axis=2)                  # [B, 255, 3549]
    xp = np.zeros((B_LOC, 255, NBLK * 128), np.float32)
    xp[:, :, :HW_TOT] = xcat
    m = {"x": np.ascontiguousarray(
        xp.reshape(B_LOC, 255, NBLK, 128).transpose(3, 0, 2, 1)
    ).reshape(128, -1)}

    gx, gy, sf, an = _cell_consts()
    pos = 0
    for (nm, H, stride), anch in zip(_SCALES, anchors):
        n = H * H
        idx = np.arange(n)
        gx[pos:pos + n] = (idx % H) * stride
        gy[pos:pos + n] = (idx // H) * stride
        sf[pos:pos + n] = stride
        an[pos:pos + n] = np.asarray(anch, np.float32)[None, :, :]
        pos += n
    cst = np.zeros((128, CST_COLS), np.float32)
    cst[:, _CST_W8:_CST_W8 + 8] = (8.0 - np.arange(8))[None, :]
    cst[:, _CST_W10:_CST_W10 + 10] = (10.0 - np.arange(10))[None, :]
    cst[:, _CST_THR] = -np.float32(np.asarray(threshold)[0])
    # [p, k] views of the packed per-cell tables, batch-replicated
    gxk = gx.reshape(NBLK, 128).T                      # [p, k]
    gyk = gy.reshape(NBLK, 128).T
    sfk = sf.reshape(NBLK, 128).T
    cst[:, _CST_GX:_CST_GX + B_LOC * NBLK] = np.tile(gxk, (1, B_LOC))
    cst[:, _CST_GY:_CST_GY + B_LOC * NBLK] = np.tile(gyk, (1, B_LOC))
    cst[:, _CST_SF:_CST_SF + B_LOC * NBLK] = np.tile(sfk, (1, B_LOC))
    cst[:, _CST_AN:_CST_AN + NBLK * 6] = (
        an.reshape(NBLK, 128, 6).transpose(1, 0, 2).reshape(128, NBLK * 6))
    m["cst"] = cst
    return m


def _assemble_core(res):
    """Interleave one core's packed outputs into reference row order."""
    o5 = res["opack"].reshape(128, B_LOC, NBLK, 3, 5)
    oc = res["opcls"].reshape(128, B_LOC, NBLK, 3, 1)
    o = np.concatenate([o5, oc], axis=4)
    flat = (o.transpose(1, 2, 0, 3, 4)
            .reshape(B_LOC, NBLK * 128, 3, 6)[:, :HW_TOT])
    per_scale = []
    pos = 0
    for _, H, _ in _SCALES:
        n = H * H
        per_scale.append(
            flat[:, pos:pos + n].reshape(B_LOC * n * 3, 6))
        pos += n
    return per_scale


def kernel(output_13, output_26, output_52, anchors_13, anchors_26,
           anchors_52, threshold):
    from concourse.bass_utils import run_bass_kernel_spmd

    nc = _get_program()
    outs = (np.asarray(output_13), np.asarray(output_26),
            np.asarray(output_52))
    anchors = (np.asarray(anchors_13), np.asarray(anchors_26),
               np.asarray(anchors_52))
    thr = np.asarray(threshold)

    in_maps = [_core_inputs(cc, outs, anchors, thr) for cc in range(N_CORES)]
    r = run_bass_kernel_spmd(nc, in_maps, list(range(N_CORES)))
    per_core = [_assemble_core(r.results[cc]) for cc in range(N_CORES)]
    blocks = []
    for si in range(3):
        blocks.append(np.concatenate([per_core[cc][si]
                                      for cc in range(N_CORES)], axis=0))
    return np.concatenate(blocks, axis=0).astype(np.float32)
